# revision 35
# baseline (speedup 1.0000x reference)
"""GQA attention (B=1, L=2048, D=2048, 32 q heads, 8 kv heads, hd=64) with RoPE,
causal mask, and output projection, on 8 Trainium2 NeuronCores.

Sharding: tensor-parallel over heads. Core c owns kv head c and q heads
4c..4c+3. Each core computes its heads' attention and a partial output
projection y_c = attn_out_c @ Wo[:, 256c:256c+256].T; the host sums the 8
partials.

v2 changes vs baseline:
  - DMA order: wq/wkv/sel/eye first, then x tiles; cos/sin tables (bf16) are
    issued after group 0's x DMAs; wo/tri after phase 1.  Removes the ~28us
    PE start stall.
  - Q-head assembly via selector matmuls on the PE (was gpsimd re-partition
    copies, ~62us).
  - Causal mask added in PSUM by a PE matmul (eye^T @ tri) issued before the
    S^T matmuls of the diagonal bank (has_written trick), replacing per-tile
    DVE adds.
  - Softmax denominator reciprocal via ScalarE exp(-ln(d)) (one table set),
    replacing the 6.5us-per-head single-lane DVE reciprocal that serialized
    the per-head pipeline.
  - bf16 operands throughout attention (RoPE outputs, K^T, V, P): 2x DVE
    modes, FWL weight loads, half the SBUF traffic.
"""

import numpy as np

L = 2048
D = 2048
HD = 64
N_HEADS = 32
N_KV = 8
NCORES = 8
QH = N_HEADS // N_KV  # q heads per core = 4
ROPE_THETA = 10000.0
NEG = -1e9

LG = 512  # ql group width
NG = L // LG  # 4 ql groups
NKT = L // 128  # 16 key tiles
NDT = D // 128  # 16 contraction tiles

_CACHE = {}


def _build_program(n_iter=1):
    import concourse.tile as tile
    import concourse.mybir as mybir
    from concourse import bacc

    f32 = mybir.dt.float32
    f16 = mybir.dt.float16
    bf16 = mybir.dt.bfloat16
    Exp = mybir.ActivationFunctionType.Exp
    Ln = mybir.ActivationFunctionType.Ln

    nc = bacc.Bacc("TRN2", target_bir_lowering=False, debug=False,
                   num_devices=NCORES)

    xT = nc.dram_tensor("xT", [128, NDT, L], f16, kind="ExternalInput")
    wq = nc.dram_tensor("wq", [128, NDT, 2 * 128], f16, kind="ExternalInput")
    wkv = nc.dram_tensor("wkv", [128, NDT, 128], f16, kind="ExternalInput")
    wo = nc.dram_tensor("wo", [128, 2, D], bf16, kind="ExternalInput")
    sel = nc.dram_tensor("sel", [128, 2 * QH, 64], bf16, kind="ExternalInput")
    cos4 = nc.dram_tensor("cos4", [128, L], bf16, kind="ExternalInput")
    sin4 = nc.dram_tensor("sin4", [128, L], bf16, kind="ExternalInput")
    cosT = nc.dram_tensor("cosT", [128, NKT, 32], bf16, kind="ExternalInput")
    sinT = nc.dram_tensor("sinT", [128, NKT, 32], bf16, kind="ExternalInput")
    tri = nc.dram_tensor("tri", [128, 128], bf16, kind="ExternalInput")
    eye = nc.dram_tensor("eye", [128, 128], bf16, kind="ExternalInput")
    y = nc.dram_tensor("y", [L, D], f16, kind="ExternalOutput")

    with tile.TileContext(nc) as tc:
        with (
            tc.tile_pool(name="consts", bufs=1) as consts,
            tc.tile_pool(name="persist", bufs=1) as persist,
            tc.tile_pool(name="r1", bufs=2) as r1p,
            tc.tile_pool(name="rb", bufs=2) as rbp,
            tc.tile_pool(name="ysb", bufs=3) as ysbp,
        ):
            # ---- weights needed for projections first ----
            # chunked; group-0 chunks interleave with its x tiles below so
            # the first matmul only waits on its own slices
            wq_sb = consts.tile([128, NDT, 256], f16)
            wkv_sb = consts.tile([128, NDT, 128], f16)
            sel_sb = consts.tile([128, 2 * QH, 64], bf16)
            eye_sb = consts.tile([128, 128], bf16)
            cos4_sb = consts.tile([128, L], bf16)
            sin4_sb = consts.tile([128, L], bf16)
            cosT_sb = consts.tile([128, NKT, 32], bf16)
            sinT_sb = consts.tile([128, NKT, 32], bf16)

            # ---- persistent intermediates ----
            qh_sb = [persist.tile([64, L], bf16, tag=f"qh{h}", name=f"qh{h}")
                     for h in range(QH)]
            kvnat = persist.tile([128, NKT, 129], bf16, tag="kvnat")
            nc.vector.memset(kvnat[:, :, 128], 1.0)
            krot = persist.tile([128, NKT, 64], bf16, tag="krot")
            kT_sb = persist.tile([64, NKT, 128], bf16, tag="kT")
            ao = [persist.tile([128, L], bf16, tag=f"ao{t}", name=f"ao{t}")
                  for t in range(2)]

            for it in range(n_iter):
                # ================= phase 1: projections =================
                with (
                    tc.tile_pool(name="xin", bufs=3) as xin,
                    tc.tile_pool(name="ropetmp", bufs=4) as ropetmp,
                    tc.tile_pool(name="qrot", bufs=4) as qrotp,
                    tc.tile_pool(name="kvtp", bufs=2) as kvtp,
                    tc.tile_pool(name="krtmp", bufs=2) as krtmp,
                    tc.tile_pool(name="proj_ps", bufs=5, space="PSUM") as proj_ps,
                    tc.tile_pool(name="tp_ps", bufs=2, space="PSUM") as tp_ps,
                    tc.tile_pool(name="qp_ps", bufs=1, space="PSUM") as qp_ps,
                ):
                    for g in range(NG):
                        gsl = slice(g * LG, (g + 1) * LG)
                        ps_qa = proj_ps.tile([128, LG], f32, tag="ps_q")
                        ps_qb = proj_ps.tile([128, LG], f32, tag="ps_q")
                        ps_kv = proj_ps.tile([128, LG], f32, tag="ps_q")
                        for ob in range(4):
                            xt = xin.tile([128, 4, LG], f16, tag="xt")
                            if it == 0 and g == 0:
                                osl = slice(4 * ob, 4 * ob + 4)
                                if ob == 0:
                                    # first k-tile alone so matmul 0 starts
                                    # after ~0.2MB instead of ~0.9MB
                                    nc.sync.dma_start(wq_sb[:, 0:1, :],
                                                      wq.ap()[:, 0:1, :])
                                    nc.sync.dma_start(wkv_sb[:, 0:1, :],
                                                      wkv.ap()[:, 0:1, :])
                                    nc.sync.dma_start(xt[:, 0:1, :],
                                                      xT.ap()[:, 0:1, gsl])
                                    nc.sync.dma_start(wq_sb[:, 1:4, :],
                                                      wq.ap()[:, 1:4, :])
                                    nc.sync.dma_start(wkv_sb[:, 1:4, :],
                                                      wkv.ap()[:, 1:4, :])
                                    nc.sync.dma_start(xt[:, 1:4, :],
                                                      xT.ap()[:, 1:4, gsl])
                                else:
                                    nc.sync.dma_start(wq_sb[:, osl, :],
                                                      wq.ap()[:, osl, :])
                                    nc.sync.dma_start(wkv_sb[:, osl, :],
                                                      wkv.ap()[:, osl, :])
                                    nc.sync.dma_start(
                                        xt[:], xT.ap()[:, 4 * ob:4 * ob + 4, gsl])
                            else:
                                nc.sync.dma_start(xt[:], xT.ap()[:, 4 * ob:4 * ob + 4, gsl])
                            for oi in range(4):
                                o = 4 * ob + oi
                                st, sp = (o == 0), (o == NDT - 1)
                                nc.tensor.matmul(ps_qa[:], wq_sb[:, o, 0:128],
                                                 xt[:, oi, :], start=st, stop=sp)
                                nc.tensor.matmul(ps_qb[:], wq_sb[:, o, 128:256],
                                                 xt[:, oi, :], start=st, stop=sp)
                                nc.tensor.matmul(ps_kv[:], wkv_sb[:, o, :],
                                                 xt[:, oi, :], start=st, stop=sp)
                        if it == 0 and g == 0:
                            # tables are first needed just below; let x/w win
                            # the DMA queue first
                            nc.sync.dma_start(cos4_sb[:], cos4.ap())
                            nc.sync.dma_start(sin4_sb[:], sin4.ap())
                            nc.sync.dma_start(sel_sb[:], sel.ap())
                            nc.sync.dma_start(eye_sb[:], eye.ap())
                            nc.sync.dma_start(cosT_sb[:], cosT.ap())
                            nc.sync.dma_start(sinT_sb[:], sinT.ap())

                        # RoPE on Q (A = tops, B = bottoms)
                        t_a = ropetmp.tile([128, LG], bf16, tag="t_a")
                        nc.vector.tensor_mul(out=t_a[:], in0=ps_qa[:], in1=cos4_sb[:, gsl])
                        t_b = ropetmp.tile([128, LG], bf16, tag="t_b")
                        nc.vector.tensor_mul(out=t_b[:], in0=ps_qb[:], in1=sin4_sb[:, gsl])
                        qa_r = qrotp.tile([128, LG], bf16, tag="qa_r")
                        nc.vector.tensor_sub(out=qa_r[:], in0=t_a[:], in1=t_b[:])
                        t_c = ropetmp.tile([128, LG], bf16, tag="t_a")
                        nc.vector.tensor_mul(out=t_c[:], in0=ps_qa[:], in1=sin4_sb[:, gsl])
                        t_d = ropetmp.tile([128, LG], bf16, tag="t_b")
                        nc.vector.tensor_mul(out=t_d[:], in0=ps_qb[:], in1=cos4_sb[:, gsl])
                        qb_r = qrotp.tile([128, LG], bf16, tag="qb_r")
                        nc.vector.tensor_add(out=qb_r[:], in0=t_c[:], in1=t_d[:])
                        # assemble per-head Q^T via selector matmuls
                        for h in range(QH):
                            qp = qp_ps.tile([64, LG], f32, tag="qp")
                            nc.tensor.matmul(qp[:], sel_sb[:, 2 * h, :], qa_r[:],
                                             start=True, stop=False)
                            nc.tensor.matmul(qp[:], sel_sb[:, 2 * h + 1, :], qb_r[:],
                                             start=False, stop=True)
                            nc.vector.tensor_copy(out=qh_sb[h][:, gsl], in_=qp[:])

                        # K/V -> natural layout; K RoPE; K back to K^T
                        kvT = kvtp.tile([128, LG], bf16, tag="kvT")
                        nc.vector.tensor_copy(out=kvT[:], in_=ps_kv[:])
                        for ki in range(4 * g, 4 * g + 4):
                            tp = tp_ps.tile([128, 128], bf16, tag="tp")
                            nc.tensor.transpose(
                                tp[:], kvT[:, (ki - 4 * g) * 128:(ki - 4 * g + 1) * 128],
                                eye_sb[:])
                            nc.vector.tensor_copy(out=kvnat[:, ki, 0:128], in_=tp[:])
                        ksl = slice(4 * g, 4 * g + 4)
                        u1 = krtmp.tile([128, 4, 32], bf16, tag="u1")
                        nc.vector.tensor_mul(out=u1[:], in0=kvnat[:, ksl, 0:32],
                                             in1=cosT_sb[:, ksl, :])
                        u2 = krtmp.tile([128, 4, 32], bf16, tag="u2")
                        nc.vector.tensor_mul(out=u2[:], in0=kvnat[:, ksl, 32:64],
                                             in1=sinT_sb[:, ksl, :])
                        nc.vector.tensor_sub(out=krot[:, ksl, 0:32], in0=u1[:], in1=u2[:])
                        u3 = krtmp.tile([128, 4, 32], bf16, tag="u1")
                        nc.vector.tensor_mul(out=u3[:], in0=kvnat[:, ksl, 0:32],
                                             in1=sinT_sb[:, ksl, :])
                        u4 = krtmp.tile([128, 4, 32], bf16, tag="u2")
                        nc.vector.tensor_mul(out=u4[:], in0=kvnat[:, ksl, 32:64],
                                             in1=cosT_sb[:, ksl, :])
                        nc.vector.tensor_add(out=krot[:, ksl, 32:64], in0=u3[:], in1=u4[:])
                        for ki in range(4 * g, 4 * g + 4):
                            tb = tp_ps.tile([128, 128], bf16, tag="tp")
                            nc.tensor.transpose(tb[0:64, :], krot[:, ki, :], eye_sb[:])
                            nc.vector.tensor_copy(out=kT_sb[:, ki, :], in_=tb[0:64, :])

                # ---- remaining constants (first used below) ----
                if it == 0:
                    wo_sb = consts.tile([128, 2, D], bf16)
                    nc.sync.dma_start(wo_sb[:], wo.ap())
                    tri_sb = consts.tile([128, 128], bf16)
                    nc.sync.dma_start(tri_sb[:], tri.ap())

                # ================= phase 2: attention + Wo ==============
                LW = 2 * LG  # 1024-wide attention groups
                with (
                    tc.tile_pool(name="pt", bufs=3) as ptp,
                    tc.tile_pool(name="st_ps", bufs=2, space="PSUM") as st_ps,
                    tc.tile_pool(name="pv_ps", bufs=1, space="PSUM") as pv_ps,
                    tc.tile_pool(name="wo_ps", bufs=2, space="PSUM") as wo_ps,
                ):
                    for j in range(L // LW):
                        jsl = slice(j * LW, (j + 1) * LW)
                        nkt = 8 * j + 8
                        # denominator rows parked at quadrant-aligned
                        # partitions (cross-partition [1,N] moves need it)
                        d4 = r1p.tile([128, LW], f32, tag="d4")
                        uao = [rbp.tile([64, LW], bf16, tag=f"uao{h}",
                                        name=f"uao{h}")
                               for h in range(QH)]
                        for h in range(QH):
                            pv = pv_ps.tile([65, LW], f32, tag="pv")
                            for ki in range(nkt):
                                # live columns of this 1024 group
                                off = max(0, 128 * ki - j * LW)
                                stp = st_ps.tile([128, LW], f32, tag="st")
                                diag = off < LW and 128 * ki >= j * LW
                                for h2 in range(2):
                                    lo = max(off, h2 * LG)
                                    if lo >= (h2 + 1) * LG:
                                        continue
                                    s2 = slice(lo, (h2 + 1) * LG)
                                    nc.tensor.matmul(
                                        stp[:, s2], kT_sb[:, ki, :],
                                        qh_sb[h][:, j * LW + lo:j * LW + (h2 + 1) * LG],
                                        start=True, stop=True)
                                pt = ptp.tile([128, LW], bf16, tag="pt")
                                nc.scalar.activation(pt[:, off:LW], stp[:, off:LW], Exp)
                                if diag:
                                    # zero the dead upper triangle of the
                                    # diagonal block after exp (0/1 mask)
                                    dsl = slice(off, off + 128)
                                    nc.vector.tensor_mul(out=pt[:, dsl],
                                                         in0=pt[:, dsl],
                                                         in1=tri_sb[:])
                                for h2 in range(2):
                                    lo = max(off, h2 * LG)
                                    if lo >= (h2 + 1) * LG:
                                        continue
                                    s2 = slice(lo, (h2 + 1) * LG)
                                    nc.tensor.matmul(
                                        pv[:, s2], kvnat[:, ki, 64:129], pt[:, s2],
                                        start=(ki == 0),
                                        stop=(ki == 8 * j + 4 * h2 + 3))
                            # drain pv fast so the next head's PV can start:
                            # unnormalized O and the denominator row
                            nc.vector.tensor_copy(out=uao[h][:], in_=pv[0:64, :])
                            nc.vector.tensor_copy(out=d4[32 * h:32 * h + 1, :],
                                                  in_=pv[64:65, :])
                            if h % 2 == 0:
                                continue
                            # pair-batched 1/d = exp(-ln d) on ScalarE, then
                            # broadcast + normalize; pair 0 finishing early
                            # lets Wo's first accumulation matmul start
                            psl = slice(32 * (h - 1), 32 * h + 1)
                            lnd = r1p.tile([128, LW], f32, tag="lnd")
                            nc.scalar.activation(lnd[psl, :], d4[psl, :], Ln)
                            r4 = r1p.tile([128, LW], f32, tag="r4")
                            nc.scalar.activation(r4[psl, :], lnd[psl, :], Exp,
                                                 scale=-1.0)
                            for h2 in (h - 1, h):
                                r1 = r1p.tile([1, LW], f32, tag="r1")
                                nc.vector.tensor_copy(
                                    out=r1[:], in_=r4[32 * h2:32 * h2 + 1, :])
                                rb = rbp.tile([64, LW], f32, tag="rb")
                                nc.gpsimd.partition_broadcast(rb[:], r1[:])
                                nc.vector.tensor_mul(
                                    out=ao[h2 // 2][64 * (h2 % 2):64 * (h2 % 2) + 64, jsl],
                                    in0=uao[h2][:], in1=rb[:])

                        # output projection for rows of this group; three
                        # of group 0's m-tiles are held back and emitted
                        # during group 1's tail normalization so the PE
                        # stays busy (and the clock warm) through it
                        if j == 0:
                            m_list = range(0, 5)
                        else:
                            m_list = list(range(5, 8)) + list(range(8, 16))
                        for m in m_list:
                            msl = slice(m * 128, (m + 1) * 128)
                            ys = ysbp.tile([128, D], f16, tag="ys")
                            for gn in range(NG):
                                nsl = slice(gn * LG, (gn + 1) * LG)
                                yp = wo_ps.tile([128, LG], f32, tag="yp")
                                nc.tensor.matmul(yp[:], ao[0][:, msl], wo_sb[:, 0, nsl],
                                                 start=True, stop=False)
                                nc.tensor.matmul(yp[:], ao[1][:, msl], wo_sb[:, 1, nsl],
                                                 start=False, stop=True)
                                nc.vector.tensor_copy(out=ys[:, nsl], in_=yp[:])
                            nc.sync.dma_start(y.ap()[msl, :], ys[:])

    nc.compile()
    return nc


def _host_prep(x, attn_scale, Wq, Wk, Wv, Wo):
    """Build the 8 per-core input maps."""
    import ml_dtypes
    bf16 = ml_dtypes.bfloat16

    xT = np.ascontiguousarray(x.reshape(L, D).T)  # [D, L]
    xT_dev = np.ascontiguousarray(xT.reshape(NDT, 128, L).transpose(1, 0, 2))

    pos = np.arange(L, dtype=np.float64)
    inv_freq = 1.0 / (ROPE_THETA ** (np.arange(0, HD, 2, dtype=np.float64) / HD))
    ang = pos[:, None] * inv_freq[None, :]  # [L, 32]
    cos = np.cos(ang).astype(np.float32)  # [L, 32]
    sin = np.sin(ang).astype(np.float32)
    cos4 = np.ascontiguousarray(np.tile(cos.T, (4, 1)))  # [128, L]
    sin4 = np.ascontiguousarray(np.tile(sin.T, (4, 1)))
    cosT = np.ascontiguousarray(cos.reshape(NKT, 128, 32).transpose(1, 0, 2))
    sinT = np.ascontiguousarray(sin.reshape(NKT, 128, 32).transpose(1, 0, 2))

    p = np.arange(128)
    tri = np.where(p[:, None] <= p[None, :], 1.0, 0.0).astype(np.float32)
    eye = np.eye(128, dtype=np.float32)

    # selector matrices: qh[h][0:32] = qa_r[32h:32h+32]; qh[h][32:64] = qb_r[...]
    sel = np.zeros((128, 2 * QH, 64), dtype=np.float32)
    for h in range(QH):
        for r in range(32):
            sel[32 * h + r, 2 * h, r] = 1.0
            sel[32 * h + r, 2 * h + 1, 32 + r] = 1.0

    kscale = float(attn_scale.reshape(-1)[0]) * HD ** -0.5

    in_maps = []
    for c in range(NCORES):
        rows_a = [Wq[256 * c + 64 * j:256 * c + 64 * j + 32] for j in range(QH)]
        rows_b = [Wq[256 * c + 64 * j + 32:256 * c + 64 * j + 64] for j in range(QH)]
        WqAB = np.concatenate(rows_a + rows_b, axis=0)  # [256, D]
        wq_dev = np.ascontiguousarray(
            WqAB.T.reshape(NDT, 128, 256).transpose(1, 0, 2))

        Wk_c = Wk[64 * c:64 * c + 64] * kscale
        Wv_c = Wv[64 * c:64 * c + 64]
        WKV = np.concatenate([Wk_c, Wv_c], axis=0)  # [128, D]
        wkv_dev = np.ascontiguousarray(
            WKV.T.reshape(NDT, 128, 128).transpose(1, 0, 2))

        WoT_c = Wo[:, 256 * c:256 * c + 256].T  # [256, D]
        wo_dev = np.ascontiguousarray(
            WoT_c.reshape(2, 128, D).transpose(1, 0, 2))

        in_maps.append({
            "xT": xT_dev.astype(np.float16), "wq": wq_dev.astype(np.float16),
            "wkv": wkv_dev.astype(np.float16), "wo": wo_dev.astype(bf16),
            "sel": sel.astype(bf16),
            "cos4": cos4.astype(bf16), "sin4": sin4.astype(bf16),
            "cosT": cosT.astype(bf16), "sinT": sinT.astype(bf16),
            "tri": tri.astype(bf16), "eye": eye.astype(bf16),
        })
    return in_maps


def _get_program(n_iter=1):
    key = f"nc{n_iter}"
    if key not in _CACHE:
        _CACHE[key] = _build_program(n_iter)
    return _CACHE[key]


def run(inputs, trace=False):
    """Run on 8 NeuronCores; returns (y_full, BassKernelResults)."""
    from concourse import bass_utils

    in_maps = _host_prep(inputs["x"], inputs["attn_scale"], inputs["Wq"],
                         inputs["Wk"], inputs["Wv"], inputs["Wo"])
    nc = _get_program()
    res = bass_utils.run_bass_kernel_spmd(
        nc, in_maps, core_ids=list(range(NCORES)), trace=trace)
    parts = np.stack([res.results[c]["y"] for c in range(NCORES)])
    y = parts.sum(axis=0, dtype=np.float64).astype(np.float32)
    return y.reshape(1, L, D), res


def kernel(**inputs):
    y, _ = run(inputs, trace=False)
    return y


# revision 36
# speedup vs baseline: 1.1958x; 1.1958x over previous
"""GQA attention (B=1, L=2048, D=2048, 32 q heads, 8 kv heads, hd=64) with RoPE,
causal mask, and output projection, on 8 Trainium2 NeuronCores.

Sharding: tensor-parallel over heads. Core c owns kv head c and q heads
4c..4c+3. Each core computes its heads' attention and a partial output
projection y_c = attn_out_c @ Wo[:, 256c:256c+256].T; the host sums the 8
partials.

v2 changes vs baseline:
  - DMA order: wq/wkv/sel/eye first, then x tiles; cos/sin tables (bf16) are
    issued after group 0's x DMAs; wo/tri after phase 1.  Removes the ~28us
    PE start stall.
  - Q-head assembly via selector matmuls on the PE (was gpsimd re-partition
    copies, ~62us).
  - Causal mask added in PSUM by a PE matmul (eye^T @ tri) issued before the
    S^T matmuls of the diagonal bank (has_written trick), replacing per-tile
    DVE adds.
  - Softmax denominator reciprocal via ScalarE exp(-ln(d)) (one table set),
    replacing the 6.5us-per-head single-lane DVE reciprocal that serialized
    the per-head pipeline.
  - bf16 operands throughout attention (RoPE outputs, K^T, V, P): 2x DVE
    modes, FWL weight loads, half the SBUF traffic.
"""

import numpy as np

L = 2048
D = 2048
HD = 64
N_HEADS = 32
N_KV = 8
NCORES = 8
QH = N_HEADS // N_KV  # q heads per core = 4
ROPE_THETA = 10000.0
NEG = -1e9

LG = 512  # ql group width
NG = L // LG  # 4 ql groups
NKT = L // 128  # 16 key tiles
NDT = D // 128  # 16 contraction tiles

_CACHE = {}


def _build_program(n_iter=1):
    import concourse.tile as tile
    import concourse.mybir as mybir
    from concourse import bacc

    f32 = mybir.dt.float32
    f16 = mybir.dt.float16
    bf16 = mybir.dt.bfloat16
    Exp = mybir.ActivationFunctionType.Exp
    Ln = mybir.ActivationFunctionType.Ln

    nc = bacc.Bacc("TRN2", target_bir_lowering=False, debug=False,
                   num_devices=NCORES)

    xT = nc.dram_tensor("xT", [128, NDT, L], f16, kind="ExternalInput")
    wq = nc.dram_tensor("wq", [128, NDT, 2 * 128], f16, kind="ExternalInput")
    wkv = nc.dram_tensor("wkv", [128, NDT, 128], f16, kind="ExternalInput")
    wo = nc.dram_tensor("wo", [128, 2, D], bf16, kind="ExternalInput")
    sel = nc.dram_tensor("sel", [128, 2 * QH, 64], bf16, kind="ExternalInput")
    cos4 = nc.dram_tensor("cos4", [128, L], bf16, kind="ExternalInput")
    sin4 = nc.dram_tensor("sin4", [128, L], bf16, kind="ExternalInput")
    cosT = nc.dram_tensor("cosT", [128, NKT, 32], bf16, kind="ExternalInput")
    sinT = nc.dram_tensor("sinT", [128, NKT, 32], bf16, kind="ExternalInput")
    tri = nc.dram_tensor("tri", [128, 128], bf16, kind="ExternalInput")
    eye = nc.dram_tensor("eye", [128, 128], bf16, kind="ExternalInput")
    y = nc.dram_tensor("y", [L, D], f16, kind="ExternalOutput")

    with tile.TileContext(nc) as tc:
        with (
            tc.tile_pool(name="consts", bufs=1) as consts,
            tc.tile_pool(name="persist", bufs=1) as persist,
            tc.tile_pool(name="r1", bufs=2) as r1p,
            tc.tile_pool(name="rb", bufs=2) as rbp,
            tc.tile_pool(name="ysb", bufs=3) as ysbp,
        ):
            # ---- weights needed for projections first ----
            # chunked; group-0 chunks interleave with its x tiles below so
            # the first matmul only waits on its own slices
            wq_sb = consts.tile([128, NDT, 256], f16)
            wkv_sb = consts.tile([128, NDT, 128], f16)
            sel_sb = consts.tile([128, 2 * QH, 64], bf16)
            eye_sb = consts.tile([128, 128], bf16)
            cos4_sb = consts.tile([128, L], bf16)
            sin4_sb = consts.tile([128, L], bf16)
            cosT_sb = consts.tile([128, NKT, 32], bf16)
            sinT_sb = consts.tile([128, NKT, 32], bf16)

            # ---- persistent intermediates ----
            qh_sb = [persist.tile([64, L], bf16, tag=f"qh{h}", name=f"qh{h}")
                     for h in range(QH)]
            kvnat = persist.tile([128, NKT, 129], bf16, tag="kvnat")
            nc.vector.memset(kvnat[:, :, 128], 1.0)
            krot = persist.tile([128, NKT, 64], bf16, tag="krot")
            kT_sb = persist.tile([64, NKT, 128], bf16, tag="kT")
            ao = [persist.tile([128, L], bf16, tag=f"ao{t}", name=f"ao{t}")
                  for t in range(2)]

            for it in range(n_iter):
                # ================= phase 1: projections =================
                with (
                    tc.tile_pool(name="xin", bufs=3) as xin,
                    tc.tile_pool(name="ropetmp", bufs=4) as ropetmp,
                    tc.tile_pool(name="qrot", bufs=4) as qrotp,
                    tc.tile_pool(name="kvtp", bufs=2) as kvtp,
                    tc.tile_pool(name="krtmp", bufs=2) as krtmp,
                    tc.tile_pool(name="proj_ps", bufs=5, space="PSUM") as proj_ps,
                    tc.tile_pool(name="tp_ps", bufs=2, space="PSUM") as tp_ps,
                    tc.tile_pool(name="qp_ps", bufs=1, space="PSUM") as qp_ps,
                ):
                    for g in range(NG):
                        gsl = slice(g * LG, (g + 1) * LG)
                        ps_qa = proj_ps.tile([128, LG], f32, tag="ps_q")
                        ps_qb = proj_ps.tile([128, LG], f32, tag="ps_q")
                        ps_kv = proj_ps.tile([128, LG], f32, tag="ps_q")
                        for ob in range(4):
                            xt = xin.tile([128, 4, LG], f16, tag="xt")
                            if it == 0 and g == 0:
                                osl = slice(4 * ob, 4 * ob + 4)
                                if ob == 0:
                                    # first k-tile alone so matmul 0 starts
                                    # after ~0.2MB instead of ~0.9MB
                                    nc.sync.dma_start(wq_sb[:, 0:1, :],
                                                      wq.ap()[:, 0:1, :])
                                    nc.sync.dma_start(wkv_sb[:, 0:1, :],
                                                      wkv.ap()[:, 0:1, :])
                                    nc.sync.dma_start(xt[:, 0:1, :],
                                                      xT.ap()[:, 0:1, gsl])
                                    nc.sync.dma_start(wq_sb[:, 1:4, :],
                                                      wq.ap()[:, 1:4, :])
                                    nc.sync.dma_start(wkv_sb[:, 1:4, :],
                                                      wkv.ap()[:, 1:4, :])
                                    nc.sync.dma_start(xt[:, 1:4, :],
                                                      xT.ap()[:, 1:4, gsl])
                                else:
                                    nc.sync.dma_start(wq_sb[:, osl, :],
                                                      wq.ap()[:, osl, :])
                                    nc.sync.dma_start(wkv_sb[:, osl, :],
                                                      wkv.ap()[:, osl, :])
                                    nc.sync.dma_start(
                                        xt[:], xT.ap()[:, 4 * ob:4 * ob + 4, gsl])
                            else:
                                nc.sync.dma_start(xt[:], xT.ap()[:, 4 * ob:4 * ob + 4, gsl])
                            for oi in range(4):
                                o = 4 * ob + oi
                                st, sp = (o == 0), (o == NDT - 1)
                                nc.tensor.matmul(ps_qa[:], wq_sb[:, o, 0:128],
                                                 xt[:, oi, :], start=st, stop=sp)
                                nc.tensor.matmul(ps_qb[:], wq_sb[:, o, 128:256],
                                                 xt[:, oi, :], start=st, stop=sp)
                                nc.tensor.matmul(ps_kv[:], wkv_sb[:, o, :],
                                                 xt[:, oi, :], start=st, stop=sp)
                        if it == 0 and g == 0:
                            # tables are first needed just below; let x/w win
                            # the DMA queue first
                            nc.sync.dma_start(cos4_sb[:], cos4.ap())
                            nc.sync.dma_start(sin4_sb[:], sin4.ap())
                            nc.sync.dma_start(sel_sb[:], sel.ap())
                            nc.sync.dma_start(eye_sb[:], eye.ap())
                            nc.sync.dma_start(cosT_sb[:], cosT.ap())
                            nc.sync.dma_start(sinT_sb[:], sinT.ap())

                        # RoPE on Q (A = tops, B = bottoms)
                        t_a = ropetmp.tile([128, LG], bf16, tag="t_a")
                        nc.vector.tensor_mul(out=t_a[:], in0=ps_qa[:], in1=cos4_sb[:, gsl])
                        t_b = ropetmp.tile([128, LG], bf16, tag="t_b")
                        nc.vector.tensor_mul(out=t_b[:], in0=ps_qb[:], in1=sin4_sb[:, gsl])
                        qa_r = qrotp.tile([128, LG], bf16, tag="qa_r")
                        nc.vector.tensor_sub(out=qa_r[:], in0=t_a[:], in1=t_b[:])
                        t_c = ropetmp.tile([128, LG], bf16, tag="t_a")
                        nc.vector.tensor_mul(out=t_c[:], in0=ps_qa[:], in1=sin4_sb[:, gsl])
                        t_d = ropetmp.tile([128, LG], bf16, tag="t_b")
                        nc.vector.tensor_mul(out=t_d[:], in0=ps_qb[:], in1=cos4_sb[:, gsl])
                        qb_r = qrotp.tile([128, LG], bf16, tag="qb_r")
                        nc.vector.tensor_add(out=qb_r[:], in0=t_c[:], in1=t_d[:])
                        # assemble per-head Q^T via selector matmuls
                        for h in range(QH):
                            qp = qp_ps.tile([64, LG], f32, tag="qp")
                            nc.tensor.matmul(qp[:], sel_sb[:, 2 * h, :], qa_r[:],
                                             start=True, stop=False)
                            nc.tensor.matmul(qp[:], sel_sb[:, 2 * h + 1, :], qb_r[:],
                                             start=False, stop=True)
                            nc.vector.tensor_copy(out=qh_sb[h][:, gsl], in_=qp[:])

                        # K/V -> natural layout; K RoPE; K back to K^T
                        kvT = kvtp.tile([128, LG], bf16, tag="kvT")
                        nc.vector.tensor_copy(out=kvT[:], in_=ps_kv[:])
                        for ki in range(4 * g, 4 * g + 4):
                            tp = tp_ps.tile([128, 128], bf16, tag="tp")
                            nc.tensor.transpose(
                                tp[:], kvT[:, (ki - 4 * g) * 128:(ki - 4 * g + 1) * 128],
                                eye_sb[:])
                            nc.vector.tensor_copy(out=kvnat[:, ki, 0:128], in_=tp[:])
                        ksl = slice(4 * g, 4 * g + 4)
                        u1 = krtmp.tile([128, 4, 32], bf16, tag="u1")
                        nc.vector.tensor_mul(out=u1[:], in0=kvnat[:, ksl, 0:32],
                                             in1=cosT_sb[:, ksl, :])
                        u2 = krtmp.tile([128, 4, 32], bf16, tag="u2")
                        nc.vector.tensor_mul(out=u2[:], in0=kvnat[:, ksl, 32:64],
                                             in1=sinT_sb[:, ksl, :])
                        nc.vector.tensor_sub(out=krot[:, ksl, 0:32], in0=u1[:], in1=u2[:])
                        u3 = krtmp.tile([128, 4, 32], bf16, tag="u1")
                        nc.vector.tensor_mul(out=u3[:], in0=kvnat[:, ksl, 0:32],
                                             in1=sinT_sb[:, ksl, :])
                        u4 = krtmp.tile([128, 4, 32], bf16, tag="u2")
                        nc.vector.tensor_mul(out=u4[:], in0=kvnat[:, ksl, 32:64],
                                             in1=cosT_sb[:, ksl, :])
                        nc.vector.tensor_add(out=krot[:, ksl, 32:64], in0=u3[:], in1=u4[:])
                        for ki in range(4 * g, 4 * g + 4):
                            tb = tp_ps.tile([128, 128], bf16, tag="tp")
                            nc.tensor.transpose(tb[0:64, :], krot[:, ki, :], eye_sb[:])
                            nc.vector.tensor_copy(out=kT_sb[:, ki, :], in_=tb[0:64, :])

                # ---- remaining constants (first used below) ----
                if it == 0:
                    wo_sb = consts.tile([128, 2, D], bf16)
                    nc.sync.dma_start(wo_sb[:], wo.ap())
                    tri_sb = consts.tile([128, 128], bf16)
                    nc.sync.dma_start(tri_sb[:], tri.ap())

                # ================= phase 2: attention + Wo ==============
                LW = 2 * LG  # 1024-wide attention groups
                with (
                    tc.tile_pool(name="pt", bufs=3) as ptp,
                    tc.tile_pool(name="st_ps", bufs=2, space="PSUM") as st_ps,
                    tc.tile_pool(name="pv_ps", bufs=1, space="PSUM") as pv_ps,
                    tc.tile_pool(name="wo_ps", bufs=2, space="PSUM") as wo_ps,
                ):
                    for j in range(L // LW):
                        jsl = slice(j * LW, (j + 1) * LW)
                        nkt = 8 * j + 8
                        # denominator rows parked at quadrant-aligned
                        # partitions (cross-partition [1,N] moves need it)
                        d4 = r1p.tile([128, LW], f32, tag="d4")
                        uao = [rbp.tile([64, LW], bf16, tag=f"uao{h}",
                                        name=f"uao{h}")
                               for h in range(QH)]
                        for h in range(QH):
                            pv = pv_ps.tile([65, LW], f32, tag="pv")
                            for ki in range(nkt):
                                # live columns of this 1024 group
                                off = max(0, 128 * ki - j * LW)
                                stp = st_ps.tile([128, LW], f32, tag="st")
                                diag = off < LW and 128 * ki >= j * LW
                                for h2 in range(2):
                                    lo = max(off, h2 * LG)
                                    if lo >= (h2 + 1) * LG:
                                        continue
                                    s2 = slice(lo, (h2 + 1) * LG)
                                    nc.tensor.matmul(
                                        stp[:, s2], kT_sb[:, ki, :],
                                        qh_sb[h][:, j * LW + lo:j * LW + (h2 + 1) * LG],
                                        start=True, stop=True)
                                pt = ptp.tile([128, LW], bf16, tag="pt")
                                nc.scalar.activation(pt[:, off:LW], stp[:, off:LW], Exp)
                                if diag:
                                    # zero the dead upper triangle of the
                                    # diagonal block after exp (0/1 mask)
                                    dsl = slice(off, off + 128)
                                    nc.vector.tensor_mul(out=pt[:, dsl],
                                                         in0=pt[:, dsl],
                                                         in1=tri_sb[:])
                                for h2 in range(2):
                                    lo = max(off, h2 * LG)
                                    if lo >= (h2 + 1) * LG:
                                        continue
                                    s2 = slice(lo, (h2 + 1) * LG)
                                    nc.tensor.matmul(
                                        pv[:, s2], kvnat[:, ki, 64:129], pt[:, s2],
                                        start=(ki == 0),
                                        stop=(ki == 8 * j + 4 * h2 + 3))
                            # drain pv fast so the next head's PV can start:
                            # unnormalized O and the denominator row
                            nc.vector.tensor_copy(out=uao[h][:], in_=pv[0:64, :])
                            nc.vector.tensor_copy(out=d4[32 * h:32 * h + 1, :],
                                                  in_=pv[64:65, :])
                            if h % 2 == 0:
                                continue
                            # pair-batched 1/d = exp(-ln d) on ScalarE, then
                            # broadcast + normalize; pair 0 finishing early
                            # lets Wo's first accumulation matmul start
                            psl = slice(32 * (h - 1), 32 * h + 1)
                            lnd = r1p.tile([128, LW], f32, tag="lnd")
                            nc.scalar.activation(lnd[psl, :], d4[psl, :], Ln)
                            r4 = r1p.tile([128, LW], f32, tag="r4")
                            nc.scalar.activation(r4[psl, :], lnd[psl, :], Exp,
                                                 scale=-1.0)
                            for h2 in (h - 1, h):
                                r1 = r1p.tile([1, LW], bf16, tag="r1")
                                nc.vector.tensor_copy(
                                    out=r1[:], in_=r4[32 * h2:32 * h2 + 1, :])
                                rb = rbp.tile([64, LW], bf16, tag="rb")
                                nc.gpsimd.partition_broadcast(rb[:], r1[:])
                                nc.vector.tensor_mul(
                                    out=ao[h2 // 2][64 * (h2 % 2):64 * (h2 % 2) + 64, jsl],
                                    in0=uao[h2][:], in1=rb[:])

                        # output projection for rows of this group; three
                        # of group 0's m-tiles are held back and emitted
                        # during group 1's tail normalization so the PE
                        # stays busy (and the clock warm) through it
                        if j == 0:
                            m_list = range(0, 5)
                        else:
                            m_list = list(range(5, 8)) + list(range(8, 16))
                        for m in m_list:
                            msl = slice(m * 128, (m + 1) * 128)
                            ys = ysbp.tile([128, D], f16, tag="ys")
                            for gn in range(NG):
                                nsl = slice(gn * LG, (gn + 1) * LG)
                                yp = wo_ps.tile([128, LG], f32, tag="yp")
                                nc.tensor.matmul(yp[:], ao[0][:, msl], wo_sb[:, 0, nsl],
                                                 start=True, stop=False)
                                nc.tensor.matmul(yp[:], ao[1][:, msl], wo_sb[:, 1, nsl],
                                                 start=False, stop=True)
                                nc.vector.tensor_copy(out=ys[:, nsl], in_=yp[:])
                            nc.sync.dma_start(y.ap()[msl, :], ys[:])

    nc.compile()
    return nc


def _host_prep(x, attn_scale, Wq, Wk, Wv, Wo):
    """Build the 8 per-core input maps."""
    import ml_dtypes
    bf16 = ml_dtypes.bfloat16

    xT = np.ascontiguousarray(x.reshape(L, D).T)  # [D, L]
    xT_dev = np.ascontiguousarray(xT.reshape(NDT, 128, L).transpose(1, 0, 2))

    pos = np.arange(L, dtype=np.float64)
    inv_freq = 1.0 / (ROPE_THETA ** (np.arange(0, HD, 2, dtype=np.float64) / HD))
    ang = pos[:, None] * inv_freq[None, :]  # [L, 32]
    cos = np.cos(ang).astype(np.float32)  # [L, 32]
    sin = np.sin(ang).astype(np.float32)
    cos4 = np.ascontiguousarray(np.tile(cos.T, (4, 1)))  # [128, L]
    sin4 = np.ascontiguousarray(np.tile(sin.T, (4, 1)))
    cosT = np.ascontiguousarray(cos.reshape(NKT, 128, 32).transpose(1, 0, 2))
    sinT = np.ascontiguousarray(sin.reshape(NKT, 128, 32).transpose(1, 0, 2))

    p = np.arange(128)
    tri = np.where(p[:, None] <= p[None, :], 1.0, 0.0).astype(np.float32)
    eye = np.eye(128, dtype=np.float32)

    # selector matrices: qh[h][0:32] = qa_r[32h:32h+32]; qh[h][32:64] = qb_r[...]
    sel = np.zeros((128, 2 * QH, 64), dtype=np.float32)
    for h in range(QH):
        for r in range(32):
            sel[32 * h + r, 2 * h, r] = 1.0
            sel[32 * h + r, 2 * h + 1, 32 + r] = 1.0

    kscale = float(attn_scale.reshape(-1)[0]) * HD ** -0.5

    in_maps = []
    for c in range(NCORES):
        rows_a = [Wq[256 * c + 64 * j:256 * c + 64 * j + 32] for j in range(QH)]
        rows_b = [Wq[256 * c + 64 * j + 32:256 * c + 64 * j + 64] for j in range(QH)]
        WqAB = np.concatenate(rows_a + rows_b, axis=0)  # [256, D]
        wq_dev = np.ascontiguousarray(
            WqAB.T.reshape(NDT, 128, 256).transpose(1, 0, 2))

        Wk_c = Wk[64 * c:64 * c + 64] * kscale
        Wv_c = Wv[64 * c:64 * c + 64]
        WKV = np.concatenate([Wk_c, Wv_c], axis=0)  # [128, D]
        wkv_dev = np.ascontiguousarray(
            WKV.T.reshape(NDT, 128, 128).transpose(1, 0, 2))

        WoT_c = Wo[:, 256 * c:256 * c + 256].T  # [256, D]
        wo_dev = np.ascontiguousarray(
            WoT_c.reshape(2, 128, D).transpose(1, 0, 2))

        in_maps.append({
            "xT": xT_dev.astype(np.float16), "wq": wq_dev.astype(np.float16),
            "wkv": wkv_dev.astype(np.float16), "wo": wo_dev.astype(bf16),
            "sel": sel.astype(bf16),
            "cos4": cos4.astype(bf16), "sin4": sin4.astype(bf16),
            "cosT": cosT.astype(bf16), "sinT": sinT.astype(bf16),
            "tri": tri.astype(bf16), "eye": eye.astype(bf16),
        })
    return in_maps


def _get_program(n_iter=1):
    key = f"nc{n_iter}"
    if key not in _CACHE:
        _CACHE[key] = _build_program(n_iter)
    return _CACHE[key]


def run(inputs, trace=False):
    """Run on 8 NeuronCores; returns (y_full, BassKernelResults)."""
    from concourse import bass_utils

    in_maps = _host_prep(inputs["x"], inputs["attn_scale"], inputs["Wq"],
                         inputs["Wk"], inputs["Wv"], inputs["Wo"])
    nc = _get_program()
    res = bass_utils.run_bass_kernel_spmd(
        nc, in_maps, core_ids=list(range(NCORES)), trace=trace)
    parts = np.stack([res.results[c]["y"] for c in range(NCORES)])
    y = parts.sum(axis=0, dtype=np.float64).astype(np.float32)
    return y.reshape(1, L, D), res


def kernel(**inputs):
    y, _ = run(inputs, trace=False)
    return y
